# revision 43
# baseline (speedup 1.0000x reference)
"""Trainium2 Bass kernel for EntityAwareLSTMLayer.

Problem (hardcoded):
  B=1024, T=365, DYN=32, STATIC=27, UNITS=256
  i_gate = sigmoid(x_static @ W_sh + bias_s)            [B, U]   (static, once)
  gx_t   = x_t @ W_ih + bias                            [B, 3U]
  gates  = gx_t + h @ W_hh                              [B, 3U]  (f|o|g)
  c      = sigmoid(f) * c + i_gate * tanh(g)
  h      = sigmoid(o) * tanh(c)
  return h_final                                        [B, U]

Sharding: data-parallel over batch, 8 cores x 128 rows. Batch rows live on
the 128 SBUF partitions; per step the gates are computed by PE matmuls
accumulating K-chunks into PSUM. Weight columns are host-reordered to
[o | f | g] so o and f share one N=512 matmul per K-chunk (one PSUM bank)
and g gets its own N=256 matmul, halving the matmul/LDWEIGHTS count vs
one matmul per gate.

The TensorE clock runs at half speed unless the engine stays busy (~3us
HAM activity window), and the recurrence stalls it every step - so the PE
stream is padded: x-contribution matmuls for future steps are issued into
the gaps, plus junk "warmer" matmuls ordered (same-engine, no semaphores)
right where the PE would otherwise idle waiting on the elementwise chain.

x_dynamic is transposed on-chip via DMA-xbar transposes of [128,128] fp16
chunks (4 timesteps per chunk); timestep t lands at partition group
32*(t%4), so W_ih is replicated at the 4 partition bases.
"""

import numpy as np

B_L = 128  # batch rows per core
T = 365
TP = 368  # T padded to a multiple of 4 for chunked transposes
DYN = 32
STATIC = 27
U = 256
NCORES = 8

WARM_A = 10  # N=128 filler matmuls after the x-matmul block

_cached = {}


def _build_program(has_bias: bool):
    from contextlib import ExitStack

    import concourse.bacc as bacc
    import concourse.masks as masks
    import concourse.tile as tile
    from concourse import mybir

    f32 = mybir.dt.float32
    f16 = mybir.dt.float16
    AF = mybir.ActivationFunctionType
    ALU = mybir.AluOpType

    nc = bacc.Bacc("TRN2", target_bir_lowering=False, debug=False)

    # weight_* arrive with columns pre-reordered to [o | f | g] (host side)
    x_dyn = nc.dram_tensor("x_dynamic", [B_L, T * DYN], f32, kind="ExternalInput")
    x_st = nc.dram_tensor("x_static", [B_L, STATIC], f32, kind="ExternalInput")
    w_ih = nc.dram_tensor("weight_ih", [DYN, 3 * U], f32, kind="ExternalInput")
    w_hh = nc.dram_tensor("weight_hh", [U, 3 * U], f32, kind="ExternalInput")
    w_sh = nc.dram_tensor("weight_sh", [STATIC, U], f32, kind="ExternalInput")
    bias = nc.dram_tensor("bias", [1, 3 * U], f32, kind="ExternalInput")
    bias_s = nc.dram_tensor("bias_s", [1, U], f32, kind="ExternalInput")
    out = nc.dram_tensor("out", [B_L, U], f32, kind="ExternalOutput")

    with tile.TileContext(nc) as tc, ExitStack() as ctx:
        const = ctx.enter_context(tc.tile_pool(name="const", bufs=1))
        xtiles = [
            const.tile([128, B_L], f16, tag=f"xt{c}", name=f"xt{c}")
            for c in range(TP // 4)
        ]
        Wih4 = const.tile([128, 3 * U], f16)  # W_ih replicated at 4 bases
        Whh0 = const.tile([128, 3 * U], f16)
        Whh1 = const.tile([128, 3 * U], f16)
        Wshb = const.tile([STATIC + 1, U], f16)  # rows 0-26 W_sh, row 27 bias_s
        xsT = const.tile([128, B_L], f16)
        ident = const.tile([128, 128], f16)
        igate = const.tile([128, U], f16)
        if has_bias:
            ones_row = const.tile([1, B_L], f16)
            bias16 = const.tile([1, 3 * U], f16)

        # [f] N=256 psum + [g|o] N=512 psum (one bank) per step: f and g
        # land early (they gate the c chain), o lands last (only needed for
        # the h products much later)
        psum_f = ctx.enter_context(tc.tile_pool(name="pf", bufs=3, space="PSUM"))
        psum_go = ctx.enter_context(tc.tile_pool(name="pgo", bufs=2, space="PSUM"))
        psum_t = ctx.enter_context(tc.tile_pool(name="pt", bufs=2, space="PSUM"))

        st = ctx.enter_context(tc.tile_pool(name="state", bufs=2))
        tmp = ctx.enter_context(tc.tile_pool(name="tmp", bufs=3))

        c_prev = st.tile([128, U], f16, tag="c")
        nc.vector.memset(c_prev[:], 0.0)
        hT0 = st.tile([128, B_L], f16, tag="h0")
        nc.vector.memset(hT0[:], 0.0)
        hT1 = st.tile([128, B_L], f16, tag="h1")
        nc.vector.memset(hT1[:], 0.0)

        with tc.tile_pool(name="stage", bufs=1) as stage:
            # --- first x_dynamic chunks FIRST: their DMA-xbar transposes
            # gate the loop's first steps, and anything issued before them
            # on the DMA fabric delays them ---
            x16 = stage.tile([B_L, TP * DYN], f16)
            nc.vector.memset(x16[:, T * DYN :], 0.0)
            NCH = 4
            split = NCH * 128
            xs32a = stage.tile([B_L, split], f32)
            nc.sync.dma_start(xs32a[:], x_dyn[:, 0:split])
            nc.vector.tensor_copy(x16[:, 0:split], xs32a[:])
            tps = []
            for c in range(NCH):
                tps.append(
                    nc.sync.dma_start_transpose(
                        xtiles[c][:], x16[:, c * 128 : (c + 1) * 128]
                    )
                )

            # --- x_static -> transposed [27, 128] + ones row 27; then
            # i_gate. This path gates the whole PE start (igate matmul ->
            # burst -> step 0), so it stages right after the x chunks ---
            xst32 = stage.tile([B_L, STATIC], f32)
            nc.sync.dma_start(xst32[:], x_st[:])
            wsh32 = stage.tile([STATIC, U], f32)
            nc.sync.dma_start(wsh32[:], w_sh[:])
            bs32 = stage.tile([1, U], f32)
            nc.sync.dma_start(bs32[:], bias_s[:])
            xst16 = stage.tile([B_L, 128], f16)
            nc.vector.memset(xst16[:], 0.0)
            nc.vector.tensor_copy(xst16[:, 0:STATIC], xst32[:])
            nc.vector.memset(xst16[:, STATIC : STATIC + 1], 1.0)
            tps.append(nc.sync.dma_start_transpose(xsT[:], xst16[:]))
            nc.vector.tensor_copy(Wshb[0:STATIC, :], wsh32[:])
            bs16 = stage.tile([1, U], f16)
            nc.vector.tensor_copy(bs16[:], bs32[:])
            # partition 27 is not engine-addressable; DMA has no such limit
            nc.sync.dma_start(Wshb[STATIC : STATIC + 1, :], bs16[:])

            wst = stage.tile([128, 3 * U], f32)
            nc.sync.dma_start(wst[:], w_hh[0:128, :])
            cpw = nc.vector.tensor_copy(Whh0[:], wst[:])
            nc.sync.dma_start(wst[:], w_hh[128:256, :])
            nc.vector.tensor_copy(Whh1[:], wst[:])
            wih32 = stage.tile([DYN, 3 * U], f32)
            nc.sync.dma_start(wih32[:], w_ih[:])
            for g in range(4):
                nc.vector.tensor_copy(Wih4[32 * g : 32 * g + 32, :], wih32[:])
            if has_bias:
                b32 = stage.tile([1, 3 * U], f32)
                nc.sync.dma_start(b32[:], bias[:])
                nc.vector.tensor_copy(bias16[:], b32[:])
                nc.vector.memset(ones_row[:], 1.0)

            masks.make_identity(nc, ident[:])

            # --- i_gate = sigmoid(x_static @ W_sh + bias_s) ---
            ig_ps = psum_f.tile([128, U], f32, tag="ps_f")
            nc.tensor.matmul(
                ig_ps[:], xsT[0 : STATIC + 1, :], Wshb[:], start=True, stop=True
            )
            nc.scalar.activation(igate[:], ig_ps[:], AF.Sigmoid)

            # --- bulk x_dynamic: ordered AFTER the first-chunk transposes
            # (and the x_static one) so its ~17us transfer doesn't block
            # them on the DMA fabric; it streams while the loop runs ---
            xs32b = stage.tile([B_L, T * DYN - split], f32)
            dmab = nc.sync.dma_start(xs32b[:], x_dyn[:, split:])
            for tp in tps:
                tile.add_dep_helper(
                    dmab.ins, tp.ins, sync=False, reason="bulk x after chunks"
                )
            nc.vector.tensor_copy(x16[:, split : T * DYN], xs32b[:])
            for c in range(NCH, TP // 4):
                nc.sync.dma_start_transpose(
                    xtiles[c][:], x16[:, c * 128 : (c + 1) * 128]
                )

        def x_f(t, ps_f):
            g4 = 32 * (t % 4)
            cc = t // 4
            xt = xtiles[cc][g4 : g4 + 32, :]
            wx = Wih4[g4 : g4 + 32, :]
            mms = [
                nc.tensor.matmul(
                    ps_f[:],
                    xt,
                    wx[:, 0:U],
                    start=True,
                    stop=False,
                    tile_position=(g4, 0),
                )
            ]
            if has_bias:
                mms.append(
                    nc.tensor.matmul(
                        ps_f[:], ones_row[:], bias16[:, 0:U], start=False, stop=False
                    )
                )
            return mms

        def x_go(t, ps_go):
            g4 = 32 * (t % 4)
            cc = t // 4
            xt = xtiles[cc][g4 : g4 + 32, :]
            wx = Wih4[g4 : g4 + 32, :]
            mms = [
                nc.tensor.matmul(
                    ps_go[:],
                    xt,
                    wx[:, U : 3 * U],
                    start=True,
                    stop=False,
                    tile_position=(g4, 0),
                )
            ]
            if has_bias:
                mms.append(
                    nc.tensor.matmul(
                        ps_go[:],
                        ones_row[:],
                        bias16[:, U : 3 * U],
                        start=False,
                        stop=False,
                    )
                )
            return mms

        # junk psum for PE warm-keeper matmuls (never read)
        junkp = ctx.enter_context(tc.tile_pool(name="junk", bufs=1, space="PSUM"))
        junk512 = junkp.tile([128, 2 * U], f32, tag="junk512")

        def warm_fill(after, n512, n128):
            prev = after
            for i in range(n512 + n128):
                nn = 2 * U if i < n512 else 128
                f = nc.tensor.matmul(
                    junk512[:, 0:nn],
                    Whh0[:, 128:256],
                    Whh0[:, 0:nn],
                    start=True,
                    stop=True,
                )
                tile.add_dep_helper(f.ins, prev.ins, sync=False, reason="warm order")
                prev = f
            return prev

        # acquire the fast PE clock once: the HAM needs ~3.4us of contiguous
        # matmul activity; a dense burst during input staging (PE idle there)
        # flips it, and steady-state pockets are too short to flip it back
        prev = cpw
        # ~3us at the cold/mid clock flips the HAM to full speed; step 0
        # follows the burst immediately in the PE stream (the igate/x paths
        # are staged early) so there is no idle gap to unflip it
        for _ in range(20):
            f = nc.tensor.matmul(
                junk512[:], ident[:], Whh0[:, 0 : 2 * U], start=True, stop=True
            )
            tile.add_dep_helper(f.ins, prev.ins, sync=False, reason="warm burst")
            prev = f

        # x contributions pre-issued ahead of the h-matmuls: f 2 steps, go 1
        f_ps = []
        for j in range(2):
            ps = psum_f.tile([128, U], f32, tag="ps_f")
            x_f(j, ps)
            f_ps.append(ps)
        go_ps = []
        ps = psum_go.tile([128, 2 * U], f32, tag="ps_go")
        x_go(0, ps)
        go_ps.append(ps)

        for t in range(T):
            last = t == T - 1
            ps_f = f_ps.pop(0)
            ps_go = go_ps.pop(0)
            # h-matmuls: [o|f] N=512 then [g] N=256 per K-chunk; explicit
            # order so sigmoid(f,o) can start after the two fo matmuls
            # order f0, g0, f1, g1, o0, o1: f and g psums complete after
            # only 4 small matmuls so sigmoid(f)/tanh(g) start early; o is
            # needed ~1.5us later by the h products
            mms = []
            mms.append(
                nc.tensor.matmul(ps_f[:], hT0[:], Whh0[:, 0:U], start=False, stop=False)
            )
            mms.append(
                nc.tensor.matmul(
                    ps_go[:, 0:U], hT0[:], Whh0[:, U : 2 * U], start=False, stop=False
                )
            )
            mms.append(
                nc.tensor.matmul(ps_f[:], hT1[:], Whh1[:, 0:U], start=False, stop=True)
            )
            mms.append(
                nc.tensor.matmul(
                    ps_go[:, 0:U], hT1[:], Whh1[:, U : 2 * U], start=False, stop=False
                )
            )
            mms.append(
                nc.tensor.matmul(
                    ps_go[:, U : 2 * U],
                    hT0[:],
                    Whh0[:, 2 * U : 3 * U],
                    start=False,
                    stop=False,
                )
            )
            mms.append(
                nc.tensor.matmul(
                    ps_go[:, U : 2 * U],
                    hT1[:],
                    Whh1[:, 2 * U : 3 * U],
                    start=False,
                    stop=True,
                )
            )
            for a, b in zip(mms[1:], mms[:-1]):
                tile.add_dep_helper(a.ins, b.ins, sync=False, reason="mm order")

            # pre-issue x matmuls (fo: t+2, g: t+1) right after the h
            # matmuls: independent work that keeps the PE stream dense
            anchor = mms[-1]
            if t + 2 < T:
                ps_n = psum_f.tile([128, U], f32, tag="ps_f")
                xmm = x_f(t + 2, ps_n)
                tile.add_dep_helper(
                    xmm[0].ins, anchor.ins, sync=False, reason="x after h"
                )
                f_ps.append(ps_n)
                anchor = xmm[-1]
            if t + 1 < T:
                ps_n = psum_go.tile([128, 2 * U], f32, tag="ps_go")
                xmm = x_go(t + 1, ps_n)
                tile.add_dep_helper(
                    xmm[0].ins, anchor.ins, sync=False, reason="xgo after xf"
                )
                go_ps.append(ps_n)
                anchor = xmm[-1]
            if not last:
                warm_fill(anchor, 5, 3)


            # sigmoid(f) first (gates the c chain), sigmoid(o) late: it is
            # only needed for the h products ~1.5us later
            sfo = tmp.tile([128, 2 * U], f16, tag="sfo")
            af = nc.scalar.activation(sfo[:, U : 2 * U], ps_f[:], AF.Sigmoid)
            tg = tmp.tile([128, U], f16, tag="tg")
            ag = nc.scalar.activation(tg[:], ps_go[:, 0:U], AF.Tanh)
            ao = nc.scalar.activation(sfo[:, 0:U], ps_go[:, U : 2 * U], AF.Sigmoid)
            tile.add_dep_helper(ag.ins, af.ins, sync=False, reason="act order")
            tile.add_dep_helper(ao.ins, ag.ins, sync=False, reason="act order")

            # c update in u-halves: two TT128 ops are cheaper than one TT256
            # and let tanh(c) half 0 start ~370ns earlier
            m1 = tmp.tile([128, U], f16, tag="m1")
            m2 = tmp.tile([128, U], f16, tag="m2")
            c_new = st.tile([128, U], f16, tag="c")
            dve = []
            for half in (0, 1):
                lo, hi = 128 * half, 128 * (half + 1)
                dve.append(
                    nc.vector.tensor_mul(m1[:, lo:hi], sfo[:, U + lo : U + hi], c_prev[:, lo:hi])
                )
            for half in (0, 1):
                lo, hi = 128 * half, 128 * (half + 1)
                dve.append(nc.vector.tensor_mul(m2[:, lo:hi], igate[:, lo:hi], tg[:, lo:hi]))
                dve.append(nc.vector.tensor_add(c_new[:, lo:hi], m1[:, lo:hi], m2[:, lo:hi]))


            if last:
                tch = tmp.tile([128, U], f32, tag="tc32")
                nc.scalar.activation(tch[:], c_new[:], AF.Tanh)
                h_out = tmp.tile([128, U], f32, tag="hout")
                nc.vector.tensor_mul(h_out[:], sfo[:, 0:U], tch[:])
                nc.sync.dma_start(out[:], h_out[:])
            else:
                # tail split into u-halves so transpose/copy/h-matmul of half 0
                # start while half 1 is still in ACT/DVE
                hTn = [None, None]
                for half in (0, 1):
                    lo, hi = 128 * half, 128 * (half + 1)
                    tch = tmp.tile([128, 128], f16, tag=f"tc{half}")
                    nc.scalar.activation(tch[:], c_new[:, lo:hi], AF.Tanh)
                    hh = tmp.tile([128, 128], f16, tag=f"hh{half}")
                    hm = nc.vector.tensor_mul(hh[:], sfo[:, lo:hi], tch[:])
                    pp = psum_t.tile([128, 128], f16, tag="pt")
                    nc.tensor.transpose(pp[:], hh[:], ident[:])
                    ht_new = st.tile([128, B_L], f16, tag=f"h{half}")
                    nc.vector.tensor_copy(
                        ht_new[:].bitcast(mybir.dt.uint32),
                        pp[:].bitcast(mybir.dt.uint32),
                    )
                    hTn[half] = ht_new
                hT0, hT1 = hTn
            c_prev = c_new

    nc.compile()
    return nc


def get_program(has_bias: bool = False):
    if has_bias not in _cached:
        _cached[has_bias] = _build_program(has_bias)
    return _cached[has_bias]


def _reorder_cols(w):
    # [f | o | g] -> [f | g | o]
    return np.concatenate([w[:, 0:U], w[:, 2 * U : 3 * U], w[:, U : 2 * U]], axis=1)


def make_in_maps(inputs):
    x_dynamic = np.asarray(inputs["x_dynamic"], dtype=np.float32)
    x_static = np.asarray(inputs["x_static"], dtype=np.float32)
    w_ih = np.ascontiguousarray(
        _reorder_cols(np.asarray(inputs["weight_ih"], dtype=np.float32))
    )
    w_hh = np.ascontiguousarray(
        _reorder_cols(np.asarray(inputs["weight_hh"], dtype=np.float32))
    )
    w_sh = np.ascontiguousarray(np.asarray(inputs["weight_sh"], dtype=np.float32))
    bias = np.ascontiguousarray(
        _reorder_cols(np.asarray(inputs["bias"], dtype=np.float32).reshape(1, 3 * U))
    )
    bias_s = np.ascontiguousarray(
        np.asarray(inputs["bias_s"], dtype=np.float32).reshape(1, U)
    )
    in_maps = []
    for i in range(NCORES):
        sl = slice(i * B_L, (i + 1) * B_L)
        in_maps.append(
            {
                "x_dynamic": np.ascontiguousarray(
                    x_dynamic[sl].reshape(B_L, T * DYN)
                ),
                "x_static": np.ascontiguousarray(x_static[sl]),
                "weight_ih": w_ih,
                "weight_hh": w_hh,
                "weight_sh": w_sh,
                "bias": bias,
                "bias_s": bias_s,
            }
        )
    return in_maps


def kernel(**inputs) -> np.ndarray:
    from concourse.bass_utils import run_bass_kernel_spmd

    has_bias = bool(np.any(np.asarray(inputs["bias"])))
    nc = get_program(has_bias)
    in_maps = make_in_maps(inputs)
    res = run_bass_kernel_spmd(nc, in_maps, core_ids=list(range(NCORES)))
    return np.concatenate([r["out"] for r in res.results], axis=0).astype(np.float32)



# revision 45
# speedup vs baseline: 1.0089x; 1.0089x over previous
"""Trainium2 Bass kernel for EntityAwareLSTMLayer.

Problem (hardcoded):
  B=1024, T=365, DYN=32, STATIC=27, UNITS=256
  i_gate = sigmoid(x_static @ W_sh + bias_s)            [B, U]   (static, once)
  gx_t   = x_t @ W_ih + bias                            [B, 3U]
  gates  = gx_t + h @ W_hh                              [B, 3U]  (f|o|g)
  c      = sigmoid(f) * c + i_gate * tanh(g)
  h      = sigmoid(o) * tanh(c)
  return h_final                                        [B, U]

Sharding: data-parallel over batch, 8 cores x 128 rows. Batch rows live on
the 128 SBUF partitions; per step the gates are computed by PE matmuls
accumulating K-chunks into PSUM. Weight columns are host-reordered to
[o | f | g] so o and f share one N=512 matmul per K-chunk (one PSUM bank)
and g gets its own N=256 matmul, halving the matmul/LDWEIGHTS count vs
one matmul per gate.

The TensorE clock runs at half speed unless the engine stays busy (~3us
HAM activity window), and the recurrence stalls it every step - so the PE
stream is padded: x-contribution matmuls for future steps are issued into
the gaps, plus junk "warmer" matmuls ordered (same-engine, no semaphores)
right where the PE would otherwise idle waiting on the elementwise chain.

x_dynamic is transposed on-chip via DMA-xbar transposes of [128,128] fp16
chunks (4 timesteps per chunk); timestep t lands at partition group
32*(t%4), so W_ih is replicated at the 4 partition bases.
"""

import numpy as np

B_L = 128  # batch rows per core
T = 365
TP = 368  # T padded to a multiple of 4 for chunked transposes
DYN = 32
STATIC = 27
U = 256
NCORES = 8

WARM_A = 10  # N=128 filler matmuls after the x-matmul block

_cached = {}


def _build_program(has_bias: bool):
    from contextlib import ExitStack

    import concourse.bacc as bacc
    import concourse.masks as masks
    import concourse.tile as tile
    from concourse import mybir

    f32 = mybir.dt.float32
    f16 = mybir.dt.float16
    AF = mybir.ActivationFunctionType
    ALU = mybir.AluOpType

    nc = bacc.Bacc("TRN2", target_bir_lowering=False, debug=False)

    # weight_* arrive with columns pre-reordered to [o | f | g] (host side)
    x_dyn = nc.dram_tensor("x_dynamic", [B_L, T * DYN], f32, kind="ExternalInput")
    x_st = nc.dram_tensor("x_static", [B_L, STATIC], f32, kind="ExternalInput")
    w_ih = nc.dram_tensor("weight_ih", [DYN, 3 * U], f32, kind="ExternalInput")
    w_hh = nc.dram_tensor("weight_hh", [U, 3 * U], f32, kind="ExternalInput")
    w_sh = nc.dram_tensor("weight_sh", [STATIC, U], f32, kind="ExternalInput")
    bias = nc.dram_tensor("bias", [1, 3 * U], f32, kind="ExternalInput")
    bias_s = nc.dram_tensor("bias_s", [1, U], f32, kind="ExternalInput")
    out = nc.dram_tensor("out", [B_L, U], f32, kind="ExternalOutput")

    with tile.TileContext(nc) as tc, ExitStack() as ctx:
        const = ctx.enter_context(tc.tile_pool(name="const", bufs=1))
        xtiles = [
            const.tile([128, B_L], f16, tag=f"xt{c}", name=f"xt{c}")
            for c in range(TP // 4)
        ]
        Wih4 = const.tile([128, 3 * U], f16)  # W_ih replicated at 4 bases
        Whh0 = const.tile([128, 3 * U], f16)
        Whh1 = const.tile([128, 3 * U], f16)
        Wshb = const.tile([STATIC + 1, U], f16)  # rows 0-26 W_sh, row 27 bias_s
        xsT = const.tile([128, B_L], f16)
        ident = const.tile([128, 128], f16)
        igate = const.tile([128, U], f16)
        if has_bias:
            ones_row = const.tile([1, B_L], f16)
            bias16 = const.tile([1, 3 * U], f16)

        # [f] N=256 psum + [g|o] N=512 psum (one bank) per step: f and g
        # land early (they gate the c chain), o lands last (only needed for
        # the h products much later)
        psum_f = ctx.enter_context(tc.tile_pool(name="pf", bufs=3, space="PSUM"))
        psum_go = ctx.enter_context(tc.tile_pool(name="pgo", bufs=2, space="PSUM"))
        psum_t = ctx.enter_context(tc.tile_pool(name="pt", bufs=2, space="PSUM"))

        st = ctx.enter_context(tc.tile_pool(name="state", bufs=2))
        tmp = ctx.enter_context(tc.tile_pool(name="tmp", bufs=3))

        c_prev = st.tile([128, U], f16, tag="c")
        nc.vector.memset(c_prev[:], 0.0)
        hT0 = st.tile([128, B_L], f16, tag="h0")
        nc.vector.memset(hT0[:], 0.0)
        hT1 = st.tile([128, B_L], f16, tag="h1")
        nc.vector.memset(hT1[:], 0.0)

        with tc.tile_pool(name="stage", bufs=1) as stage:
            # --- first x_dynamic chunks FIRST: their DMA-xbar transposes
            # gate the loop's first steps, and anything issued before them
            # on the DMA fabric delays them ---
            x16 = stage.tile([B_L, TP * DYN], f16)
            nc.vector.memset(x16[:, T * DYN :], 0.0)
            NCH = 4
            split = NCH * 128
            xs32a = stage.tile([B_L, split], f32)
            nc.sync.dma_start(xs32a[:], x_dyn[:, 0:split])
            nc.vector.tensor_copy(x16[:, 0:split], xs32a[:])
            tps = []
            for c in range(NCH):
                tps.append(
                    nc.sync.dma_start_transpose(
                        xtiles[c][:], x16[:, c * 128 : (c + 1) * 128]
                    )
                )

            wst = stage.tile([128, 3 * U], f32)
            nc.sync.dma_start(wst[:], w_hh[0:128, :])
            cpw = nc.vector.tensor_copy(Whh0[:], wst[:])
            nc.sync.dma_start(wst[:], w_hh[128:256, :])
            nc.vector.tensor_copy(Whh1[:], wst[:])
            wih32 = stage.tile([DYN, 3 * U], f32)
            nc.sync.dma_start(wih32[:], w_ih[:])
            for g in range(4):
                nc.vector.tensor_copy(Wih4[32 * g : 32 * g + 32, :], wih32[:])
            wsh32 = stage.tile([STATIC, U], f32)
            nc.sync.dma_start(wsh32[:], w_sh[:])
            nc.vector.tensor_copy(Wshb[0:STATIC, :], wsh32[:])
            bs32 = stage.tile([1, U], f32)
            nc.sync.dma_start(bs32[:], bias_s[:])
            bs16 = stage.tile([1, U], f16)
            nc.vector.tensor_copy(bs16[:], bs32[:])
            # partition 27 is not engine-addressable; DMA has no such limit
            nc.sync.dma_start(Wshb[STATIC : STATIC + 1, :], bs16[:])
            if has_bias:
                b32 = stage.tile([1, 3 * U], f32)
                nc.sync.dma_start(b32[:], bias[:])
                nc.vector.tensor_copy(bias16[:], b32[:])
                nc.vector.memset(ones_row[:], 1.0)

            # --- x_static -> transposed [27, 128] + ones row 27 ---
            xst32 = stage.tile([B_L, STATIC], f32)
            nc.sync.dma_start(xst32[:], x_st[:])
            xst16 = stage.tile([B_L, 128], f16)
            nc.vector.memset(xst16[:], 0.0)
            nc.vector.tensor_copy(xst16[:, 0:STATIC], xst32[:])
            nc.vector.memset(xst16[:, STATIC : STATIC + 1], 1.0)
            tps.append(nc.sync.dma_start_transpose(xsT[:], xst16[:]))

            masks.make_identity(nc, ident[:])

            # --- i_gate = sigmoid(x_static @ W_sh + bias_s) ---
            ig_ps = psum_f.tile([128, U], f32, tag="ps_f")
            nc.tensor.matmul(
                ig_ps[:], xsT[0 : STATIC + 1, :], Wshb[:], start=True, stop=True
            )
            nc.scalar.activation(igate[:], ig_ps[:], AF.Sigmoid)

            # --- bulk x_dynamic: ordered AFTER the first-chunk transposes
            # (and the x_static one) so its ~17us transfer doesn't block
            # them on the DMA fabric; it streams while the loop runs ---
            xs32b = stage.tile([B_L, T * DYN - split], f32)
            dmab = nc.sync.dma_start(xs32b[:], x_dyn[:, split:])
            for tp in tps:
                tile.add_dep_helper(
                    dmab.ins, tp.ins, sync=False, reason="bulk x after chunks"
                )
            nc.vector.tensor_copy(x16[:, split : T * DYN], xs32b[:])
            for c in range(NCH, TP // 4):
                nc.sync.dma_start_transpose(
                    xtiles[c][:], x16[:, c * 128 : (c + 1) * 128]
                )

        def x_f(t, ps_f):
            g4 = 32 * (t % 4)
            cc = t // 4
            xt = xtiles[cc][g4 : g4 + 32, :]
            wx = Wih4[g4 : g4 + 32, :]
            mms = [
                nc.tensor.matmul(
                    ps_f[:],
                    xt,
                    wx[:, 0:U],
                    start=True,
                    stop=False,
                    tile_position=(g4, 0),
                )
            ]
            if has_bias:
                mms.append(
                    nc.tensor.matmul(
                        ps_f[:], ones_row[:], bias16[:, 0:U], start=False, stop=False
                    )
                )
            return mms

        def x_go(t, ps_go):
            g4 = 32 * (t % 4)
            cc = t // 4
            xt = xtiles[cc][g4 : g4 + 32, :]
            wx = Wih4[g4 : g4 + 32, :]
            mms = [
                nc.tensor.matmul(
                    ps_go[:],
                    xt,
                    wx[:, U : 3 * U],
                    start=True,
                    stop=False,
                    tile_position=(g4, 0),
                )
            ]
            if has_bias:
                mms.append(
                    nc.tensor.matmul(
                        ps_go[:],
                        ones_row[:],
                        bias16[:, U : 3 * U],
                        start=False,
                        stop=False,
                    )
                )
            return mms

        # junk psum for PE warm-keeper matmuls (never read)
        junkp = ctx.enter_context(tc.tile_pool(name="junk", bufs=1, space="PSUM"))
        junk512 = junkp.tile([128, 2 * U], f32, tag="junk512")

        def warm_fill(after, n512, n128):
            prev = after
            for i in range(n512 + n128):
                nn = 2 * U if i < n512 else 128
                f = nc.tensor.matmul(
                    junk512[:, 0:nn],
                    Whh0[:, 128:256],
                    Whh0[:, 0:nn],
                    start=True,
                    stop=True,
                )
                tile.add_dep_helper(f.ins, prev.ins, sync=False, reason="warm order")
                prev = f
            return prev

        # acquire the fast PE clock once: the HAM needs ~3.4us of contiguous
        # matmul activity; a dense burst during input staging (PE idle there)
        # flips it, and steady-state pockets are too short to flip it back
        prev = cpw
        # 16 ops ≈ 3us at the cold/mid clock — enough contiguous activity to
        # flip the HAM to full speed without serializing ~19us of junk in
        # front of the first step's matmuls
        for _ in range(16):
            f = nc.tensor.matmul(
                junk512[:], ident[:], Whh0[:, 0 : 2 * U], start=True, stop=True
            )
            tile.add_dep_helper(f.ins, prev.ins, sync=False, reason="warm burst")
            prev = f

        # x contributions pre-issued ahead of the h-matmuls: f 2 steps, go 1
        f_ps = []
        for j in range(2):
            ps = psum_f.tile([128, U], f32, tag="ps_f")
            x_f(j, ps)
            f_ps.append(ps)
        go_ps = []
        ps = psum_go.tile([128, 2 * U], f32, tag="ps_go")
        x_go(0, ps)
        go_ps.append(ps)

        for t in range(T):
            last = t == T - 1
            ps_f = f_ps.pop(0)
            ps_go = go_ps.pop(0)
            # h-matmuls: [o|f] N=512 then [g] N=256 per K-chunk; explicit
            # order so sigmoid(f,o) can start after the two fo matmuls
            # order f0, g0, f1, g1, o0, o1: f and g psums complete after
            # only 4 small matmuls so sigmoid(f)/tanh(g) start early; o is
            # needed ~1.5us later by the h products
            mms = []
            mms.append(
                nc.tensor.matmul(ps_f[:], hT0[:], Whh0[:, 0:U], start=False, stop=False)
            )
            mms.append(
                nc.tensor.matmul(
                    ps_go[:, 0:U], hT0[:], Whh0[:, U : 2 * U], start=False, stop=False
                )
            )
            mms.append(
                nc.tensor.matmul(ps_f[:], hT1[:], Whh1[:, 0:U], start=False, stop=True)
            )
            mms.append(
                nc.tensor.matmul(
                    ps_go[:, 0:U], hT1[:], Whh1[:, U : 2 * U], start=False, stop=False
                )
            )
            mms.append(
                nc.tensor.matmul(
                    ps_go[:, U : 2 * U],
                    hT0[:],
                    Whh0[:, 2 * U : 3 * U],
                    start=False,
                    stop=False,
                )
            )
            mms.append(
                nc.tensor.matmul(
                    ps_go[:, U : 2 * U],
                    hT1[:],
                    Whh1[:, 2 * U : 3 * U],
                    start=False,
                    stop=True,
                )
            )
            for a, b in zip(mms[1:], mms[:-1]):
                tile.add_dep_helper(a.ins, b.ins, sync=False, reason="mm order")

            # pre-issue x matmuls (fo: t+2, g: t+1) right after the h
            # matmuls: independent work that keeps the PE stream dense
            anchor = mms[-1]
            if t + 2 < T:
                ps_n = psum_f.tile([128, U], f32, tag="ps_f")
                xmm = x_f(t + 2, ps_n)
                tile.add_dep_helper(
                    xmm[0].ins, anchor.ins, sync=False, reason="x after h"
                )
                f_ps.append(ps_n)
                anchor = xmm[-1]
            if t + 1 < T:
                ps_n = psum_go.tile([128, 2 * U], f32, tag="ps_go")
                xmm = x_go(t + 1, ps_n)
                tile.add_dep_helper(
                    xmm[0].ins, anchor.ins, sync=False, reason="xgo after xf"
                )
                go_ps.append(ps_n)
                anchor = xmm[-1]
            if not last:
                warm_fill(anchor, 5, 3)


            # sigmoid(f) first (gates the c chain), sigmoid(o) late: it is
            # only needed for the h products ~1.5us later
            sfo = tmp.tile([128, 2 * U], f16, tag="sfo")
            af = nc.scalar.activation(sfo[:, U : 2 * U], ps_f[:], AF.Sigmoid)
            tg = tmp.tile([128, U], f16, tag="tg")
            ag = nc.scalar.activation(tg[:], ps_go[:, 0:U], AF.Tanh)
            ao = nc.scalar.activation(sfo[:, 0:U], ps_go[:, U : 2 * U], AF.Sigmoid)
            tile.add_dep_helper(ag.ins, af.ins, sync=False, reason="act order")
            tile.add_dep_helper(ao.ins, ag.ins, sync=False, reason="act order")

            # c update in u-halves: two TT128 ops are cheaper than one TT256
            # and let tanh(c) half 0 start ~370ns earlier
            m1 = tmp.tile([128, U], f16, tag="m1")
            m2 = tmp.tile([128, U], f16, tag="m2")
            c_new = st.tile([128, U], f16, tag="c")
            dve = []
            for half in (0, 1):
                lo, hi = 128 * half, 128 * (half + 1)
                dve.append(
                    nc.vector.tensor_mul(m1[:, lo:hi], sfo[:, U + lo : U + hi], c_prev[:, lo:hi])
                )
            for half in (0, 1):
                lo, hi = 128 * half, 128 * (half + 1)
                dve.append(nc.vector.tensor_mul(m2[:, lo:hi], igate[:, lo:hi], tg[:, lo:hi]))
                dve.append(nc.vector.tensor_add(c_new[:, lo:hi], m1[:, lo:hi], m2[:, lo:hi]))


            if last:
                tch = tmp.tile([128, U], f32, tag="tc32")
                nc.scalar.activation(tch[:], c_new[:], AF.Tanh)
                h_out = tmp.tile([128, U], f32, tag="hout")
                nc.vector.tensor_mul(h_out[:], sfo[:, 0:U], tch[:])
                nc.sync.dma_start(out[:], h_out[:])
            else:
                # tail split into u-halves so transpose/copy/h-matmul of half 0
                # start while half 1 is still in ACT/DVE
                hTn = [None, None]
                for half in (0, 1):
                    lo, hi = 128 * half, 128 * (half + 1)
                    tch = tmp.tile([128, 128], f16, tag=f"tc{half}")
                    nc.scalar.activation(tch[:], c_new[:, lo:hi], AF.Tanh)
                    hh = tmp.tile([128, 128], f16, tag=f"hh{half}")
                    hm = nc.vector.tensor_mul(hh[:], sfo[:, lo:hi], tch[:])
                    pp = psum_t.tile([128, 128], f16, tag="pt")
                    nc.tensor.transpose(pp[:], hh[:], ident[:])
                    ht_new = st.tile([128, B_L], f16, tag=f"h{half}")
                    nc.vector.tensor_copy(
                        ht_new[:].bitcast(mybir.dt.uint32),
                        pp[:].bitcast(mybir.dt.uint32),
                    )
                    hTn[half] = ht_new
                hT0, hT1 = hTn
            c_prev = c_new

    nc.compile()
    return nc


def get_program(has_bias: bool = False):
    if has_bias not in _cached:
        _cached[has_bias] = _build_program(has_bias)
    return _cached[has_bias]


def _reorder_cols(w):
    # [f | o | g] -> [f | g | o]
    return np.concatenate([w[:, 0:U], w[:, 2 * U : 3 * U], w[:, U : 2 * U]], axis=1)


def make_in_maps(inputs):
    x_dynamic = np.asarray(inputs["x_dynamic"], dtype=np.float32)
    x_static = np.asarray(inputs["x_static"], dtype=np.float32)
    w_ih = np.ascontiguousarray(
        _reorder_cols(np.asarray(inputs["weight_ih"], dtype=np.float32))
    )
    w_hh = np.ascontiguousarray(
        _reorder_cols(np.asarray(inputs["weight_hh"], dtype=np.float32))
    )
    w_sh = np.ascontiguousarray(np.asarray(inputs["weight_sh"], dtype=np.float32))
    bias = np.ascontiguousarray(
        _reorder_cols(np.asarray(inputs["bias"], dtype=np.float32).reshape(1, 3 * U))
    )
    bias_s = np.ascontiguousarray(
        np.asarray(inputs["bias_s"], dtype=np.float32).reshape(1, U)
    )
    in_maps = []
    for i in range(NCORES):
        sl = slice(i * B_L, (i + 1) * B_L)
        in_maps.append(
            {
                "x_dynamic": np.ascontiguousarray(
                    x_dynamic[sl].reshape(B_L, T * DYN)
                ),
                "x_static": np.ascontiguousarray(x_static[sl]),
                "weight_ih": w_ih,
                "weight_hh": w_hh,
                "weight_sh": w_sh,
                "bias": bias,
                "bias_s": bias_s,
            }
        )
    return in_maps


def kernel(**inputs) -> np.ndarray:
    from concourse.bass_utils import run_bass_kernel_spmd

    has_bias = bool(np.any(np.asarray(inputs["bias"])))
    nc = get_program(has_bias)
    in_maps = make_in_maps(inputs)
    res = run_bass_kernel_spmd(nc, in_maps, core_ids=list(range(NCORES)))
    return np.concatenate([r["out"] for r in res.results], axis=0).astype(np.float32)



# revision 47
# speedup vs baseline: 1.0121x; 1.0031x over previous
"""Trainium2 Bass kernel for EntityAwareLSTMLayer.

Problem (hardcoded):
  B=1024, T=365, DYN=32, STATIC=27, UNITS=256
  i_gate = sigmoid(x_static @ W_sh + bias_s)            [B, U]   (static, once)
  gx_t   = x_t @ W_ih + bias                            [B, 3U]
  gates  = gx_t + h @ W_hh                              [B, 3U]  (f|o|g)
  c      = sigmoid(f) * c + i_gate * tanh(g)
  h      = sigmoid(o) * tanh(c)
  return h_final                                        [B, U]

Sharding: data-parallel over batch, 8 cores x 128 rows. Batch rows live on
the 128 SBUF partitions; per step the gates are computed by PE matmuls
accumulating K-chunks into PSUM. Weight columns are host-reordered to
[o | f | g] so o and f share one N=512 matmul per K-chunk (one PSUM bank)
and g gets its own N=256 matmul, halving the matmul/LDWEIGHTS count vs
one matmul per gate.

The TensorE clock runs at half speed unless the engine stays busy (~3us
HAM activity window), and the recurrence stalls it every step - so the PE
stream is padded: x-contribution matmuls for future steps are issued into
the gaps, plus junk "warmer" matmuls ordered (same-engine, no semaphores)
right where the PE would otherwise idle waiting on the elementwise chain.

x_dynamic is transposed on-chip via DMA-xbar transposes of [128,128] fp16
chunks (4 timesteps per chunk); timestep t lands at partition group
32*(t%4), so W_ih is replicated at the 4 partition bases.
"""

import numpy as np

B_L = 128  # batch rows per core
T = 365
TP = 368  # T padded to a multiple of 4 for chunked transposes
DYN = 32
STATIC = 27
U = 256
NCORES = 8

WARM_A = 10  # N=128 filler matmuls after the x-matmul block

_cached = {}


def _build_program(has_bias: bool):
    from contextlib import ExitStack

    import concourse.bacc as bacc
    import concourse.masks as masks
    import concourse.tile as tile
    from concourse import mybir

    f32 = mybir.dt.float32
    f16 = mybir.dt.float16
    AF = mybir.ActivationFunctionType
    ALU = mybir.AluOpType

    nc = bacc.Bacc("TRN2", target_bir_lowering=False, debug=False)

    # weight_* arrive with columns pre-reordered to [o | f | g] (host side)
    x_dyn = nc.dram_tensor("x_dynamic", [B_L, T * DYN], f32, kind="ExternalInput")
    x_st = nc.dram_tensor("x_static", [B_L, STATIC], f32, kind="ExternalInput")
    w_ih = nc.dram_tensor("weight_ih", [DYN, 3 * U], f32, kind="ExternalInput")
    w_hh = nc.dram_tensor("weight_hh", [U, 3 * U], f32, kind="ExternalInput")
    w_sh = nc.dram_tensor("weight_sh", [STATIC, U], f32, kind="ExternalInput")
    bias = nc.dram_tensor("bias", [1, 3 * U], f32, kind="ExternalInput")
    bias_s = nc.dram_tensor("bias_s", [1, U], f32, kind="ExternalInput")
    out = nc.dram_tensor("out", [B_L, U], f32, kind="ExternalOutput")

    with tile.TileContext(nc) as tc, ExitStack() as ctx:
        const = ctx.enter_context(tc.tile_pool(name="const", bufs=1))
        xtiles = [
            const.tile([128, B_L], f16, tag=f"xt{c}", name=f"xt{c}")
            for c in range(TP // 4)
        ]
        Wih4 = const.tile([128, 3 * U], f16)  # W_ih replicated at 4 bases
        Whh0 = const.tile([128, 3 * U], f16)
        Whh1 = const.tile([128, 3 * U], f16)
        Wshb = const.tile([STATIC + 1, U], f16)  # rows 0-26 W_sh, row 27 bias_s
        xsT = const.tile([128, B_L], f16)
        ident = const.tile([128, 128], f16)
        igate = const.tile([128, U], f16)
        if has_bias:
            ones_row = const.tile([1, B_L], f16)
            bias16 = const.tile([1, 3 * U], f16)

        # [f] N=256 psum + [g|o] N=512 psum (one bank) per step: f and g
        # land early (they gate the c chain), o lands last (only needed for
        # the h products much later)
        psum_f = ctx.enter_context(tc.tile_pool(name="pf", bufs=3, space="PSUM"))
        psum_go = ctx.enter_context(tc.tile_pool(name="pgo", bufs=2, space="PSUM"))
        psum_t = ctx.enter_context(tc.tile_pool(name="pt", bufs=2, space="PSUM"))

        st = ctx.enter_context(tc.tile_pool(name="state", bufs=2))
        tmp = ctx.enter_context(tc.tile_pool(name="tmp", bufs=3))

        c_prev = st.tile([128, U], f16, tag="c")
        nc.vector.memset(c_prev[:], 0.0)
        hT0 = st.tile([128, B_L], f16, tag="h0")
        nc.vector.memset(hT0[:], 0.0)
        hT1 = st.tile([128, B_L], f16, tag="h1")
        nc.vector.memset(hT1[:], 0.0)

        with tc.tile_pool(name="stage", bufs=1) as stage:
            # SP/DMA issue order matters: each DMA-xbar transpose occupies
            # ~1.25us of issue, so the weight + x_static loads (which gate
            # the igate matmul and thus the whole PE start) go first; the
            # x chunks (not needed until ~20us) load early but transpose
            # after; the 17us bulk x transfer is pinned last
            x16 = stage.tile([B_L, TP * DYN], f16)
            nc.vector.memset(x16[:, T * DYN :], 0.0)
            NCH = 4
            split = NCH * 128
            xs32a = stage.tile([B_L, split], f32)
            nc.sync.dma_start(xs32a[:], x_dyn[:, 0:split])
            nc.vector.tensor_copy(x16[:, 0:split], xs32a[:])
            tps = []

            wst = stage.tile([128, 3 * U], f32)
            nc.sync.dma_start(wst[:], w_hh[0:128, :])
            cpw = nc.vector.tensor_copy(Whh0[:], wst[:])
            nc.sync.dma_start(wst[:], w_hh[128:256, :])
            nc.vector.tensor_copy(Whh1[:], wst[:])
            wih32 = stage.tile([DYN, 3 * U], f32)
            nc.sync.dma_start(wih32[:], w_ih[:])
            for g in range(4):
                nc.vector.tensor_copy(Wih4[32 * g : 32 * g + 32, :], wih32[:])
            wsh32 = stage.tile([STATIC, U], f32)
            nc.sync.dma_start(wsh32[:], w_sh[:])
            nc.vector.tensor_copy(Wshb[0:STATIC, :], wsh32[:])
            bs32 = stage.tile([1, U], f32)
            nc.sync.dma_start(bs32[:], bias_s[:])
            bs16 = stage.tile([1, U], f16)
            nc.vector.tensor_copy(bs16[:], bs32[:])
            # partition 27 is not engine-addressable; DMA has no such limit
            nc.sync.dma_start(Wshb[STATIC : STATIC + 1, :], bs16[:])
            if has_bias:
                b32 = stage.tile([1, 3 * U], f32)
                nc.sync.dma_start(b32[:], bias[:])
                nc.vector.tensor_copy(bias16[:], b32[:])
                nc.vector.memset(ones_row[:], 1.0)

            # --- x_static -> transposed [27, 128] + ones row 27 ---
            xst32 = stage.tile([B_L, STATIC], f32)
            nc.sync.dma_start(xst32[:], x_st[:])
            xst16 = stage.tile([B_L, 128], f16)
            nc.vector.memset(xst16[:], 0.0)
            nc.vector.tensor_copy(xst16[:, 0:STATIC], xst32[:])
            nc.vector.memset(xst16[:, STATIC : STATIC + 1], 1.0)
            tps.append(nc.sync.dma_start_transpose(xsT[:], xst16[:]))
            for c in range(NCH):
                tp = nc.sync.dma_start_transpose(
                    xtiles[c][:], x16[:, c * 128 : (c + 1) * 128]
                )
                tile.add_dep_helper(
                    tp.ins, tps[-1].ins, sync=False, reason="chunks after xsT"
                )
                tps.append(tp)

            masks.make_identity(nc, ident[:])

            # --- i_gate = sigmoid(x_static @ W_sh + bias_s) ---
            ig_ps = psum_f.tile([128, U], f32, tag="ps_f")
            nc.tensor.matmul(
                ig_ps[:], xsT[0 : STATIC + 1, :], Wshb[:], start=True, stop=True
            )
            nc.scalar.activation(igate[:], ig_ps[:], AF.Sigmoid)

            # --- bulk x_dynamic: ordered AFTER the first-chunk transposes
            # (and the x_static one) so its ~17us transfer doesn't block
            # them on the DMA fabric; it streams while the loop runs ---
            xs32b = stage.tile([B_L, T * DYN - split], f32)
            dmab = nc.sync.dma_start(xs32b[:], x_dyn[:, split:])
            for tp in tps:
                tile.add_dep_helper(
                    dmab.ins, tp.ins, sync=False, reason="bulk x after chunks"
                )
            nc.vector.tensor_copy(x16[:, split : T * DYN], xs32b[:])
            for c in range(NCH, TP // 4):
                nc.sync.dma_start_transpose(
                    xtiles[c][:], x16[:, c * 128 : (c + 1) * 128]
                )

        def x_f(t, ps_f):
            g4 = 32 * (t % 4)
            cc = t // 4
            xt = xtiles[cc][g4 : g4 + 32, :]
            wx = Wih4[g4 : g4 + 32, :]
            mms = [
                nc.tensor.matmul(
                    ps_f[:],
                    xt,
                    wx[:, 0:U],
                    start=True,
                    stop=False,
                    tile_position=(g4, 0),
                )
            ]
            if has_bias:
                mms.append(
                    nc.tensor.matmul(
                        ps_f[:], ones_row[:], bias16[:, 0:U], start=False, stop=False
                    )
                )
            return mms

        def x_go(t, ps_go):
            g4 = 32 * (t % 4)
            cc = t // 4
            xt = xtiles[cc][g4 : g4 + 32, :]
            wx = Wih4[g4 : g4 + 32, :]
            mms = [
                nc.tensor.matmul(
                    ps_go[:],
                    xt,
                    wx[:, U : 3 * U],
                    start=True,
                    stop=False,
                    tile_position=(g4, 0),
                )
            ]
            if has_bias:
                mms.append(
                    nc.tensor.matmul(
                        ps_go[:],
                        ones_row[:],
                        bias16[:, U : 3 * U],
                        start=False,
                        stop=False,
                    )
                )
            return mms

        # junk psum for PE warm-keeper matmuls (never read)
        junkp = ctx.enter_context(tc.tile_pool(name="junk", bufs=1, space="PSUM"))
        junk512 = junkp.tile([128, 2 * U], f32, tag="junk512")

        def warm_fill(after, n512, n128):
            prev = after
            for i in range(n512 + n128):
                nn = 2 * U if i < n512 else 128
                f = nc.tensor.matmul(
                    junk512[:, 0:nn],
                    Whh0[:, 128:256],
                    Whh0[:, 0:nn],
                    start=True,
                    stop=True,
                )
                tile.add_dep_helper(f.ins, prev.ins, sync=False, reason="warm order")
                prev = f
            return prev

        # acquire the fast PE clock once: the HAM needs ~3.4us of contiguous
        # matmul activity; a dense burst during input staging (PE idle there)
        # flips it, and steady-state pockets are too short to flip it back
        prev = cpw
        # 16 ops ≈ 3us at the cold/mid clock — enough contiguous activity to
        # flip the HAM to full speed without serializing ~19us of junk in
        # front of the first step's matmuls
        for _ in range(16):
            f = nc.tensor.matmul(
                junk512[:], ident[:], Whh0[:, 0 : 2 * U], start=True, stop=True
            )
            tile.add_dep_helper(f.ins, prev.ins, sync=False, reason="warm burst")
            prev = f

        # x contributions pre-issued ahead of the h-matmuls: f 2 steps, go 1
        f_ps = []
        for j in range(2):
            ps = psum_f.tile([128, U], f32, tag="ps_f")
            x_f(j, ps)
            f_ps.append(ps)
        go_ps = []
        ps = psum_go.tile([128, 2 * U], f32, tag="ps_go")
        x_go(0, ps)
        go_ps.append(ps)

        for t in range(T):
            last = t == T - 1
            ps_f = f_ps.pop(0)
            ps_go = go_ps.pop(0)
            # h-matmuls: [o|f] N=512 then [g] N=256 per K-chunk; explicit
            # order so sigmoid(f,o) can start after the two fo matmuls
            # order f0, g0, f1, g1, o0, o1: f and g psums complete after
            # only 4 small matmuls so sigmoid(f)/tanh(g) start early; o is
            # needed ~1.5us later by the h products
            mms = []
            mms.append(
                nc.tensor.matmul(ps_f[:], hT0[:], Whh0[:, 0:U], start=False, stop=False)
            )
            mms.append(
                nc.tensor.matmul(
                    ps_go[:, 0:U], hT0[:], Whh0[:, U : 2 * U], start=False, stop=False
                )
            )
            mms.append(
                nc.tensor.matmul(ps_f[:], hT1[:], Whh1[:, 0:U], start=False, stop=True)
            )
            mms.append(
                nc.tensor.matmul(
                    ps_go[:, 0:U], hT1[:], Whh1[:, U : 2 * U], start=False, stop=False
                )
            )
            mms.append(
                nc.tensor.matmul(
                    ps_go[:, U : 2 * U],
                    hT0[:],
                    Whh0[:, 2 * U : 3 * U],
                    start=False,
                    stop=False,
                )
            )
            mms.append(
                nc.tensor.matmul(
                    ps_go[:, U : 2 * U],
                    hT1[:],
                    Whh1[:, 2 * U : 3 * U],
                    start=False,
                    stop=True,
                )
            )
            for a, b in zip(mms[1:], mms[:-1]):
                tile.add_dep_helper(a.ins, b.ins, sync=False, reason="mm order")

            # pre-issue x matmuls (fo: t+2, g: t+1) right after the h
            # matmuls: independent work that keeps the PE stream dense
            anchor = mms[-1]
            if t + 2 < T:
                ps_n = psum_f.tile([128, U], f32, tag="ps_f")
                xmm = x_f(t + 2, ps_n)
                tile.add_dep_helper(
                    xmm[0].ins, anchor.ins, sync=False, reason="x after h"
                )
                f_ps.append(ps_n)
                anchor = xmm[-1]
            if t + 1 < T:
                ps_n = psum_go.tile([128, 2 * U], f32, tag="ps_go")
                xmm = x_go(t + 1, ps_n)
                tile.add_dep_helper(
                    xmm[0].ins, anchor.ins, sync=False, reason="xgo after xf"
                )
                go_ps.append(ps_n)
                anchor = xmm[-1]
            if not last:
                warm_fill(anchor, 5, 3)


            # sigmoid(f) first (gates the c chain), sigmoid(o) late: it is
            # only needed for the h products ~1.5us later
            sfo = tmp.tile([128, 2 * U], f16, tag="sfo")
            af = nc.scalar.activation(sfo[:, U : 2 * U], ps_f[:], AF.Sigmoid)
            tg = tmp.tile([128, U], f16, tag="tg")
            ag = nc.scalar.activation(tg[:], ps_go[:, 0:U], AF.Tanh)
            ao = nc.scalar.activation(sfo[:, 0:U], ps_go[:, U : 2 * U], AF.Sigmoid)
            tile.add_dep_helper(ag.ins, af.ins, sync=False, reason="act order")
            tile.add_dep_helper(ao.ins, ag.ins, sync=False, reason="act order")

            # c update in u-halves: two TT128 ops are cheaper than one TT256
            # and let tanh(c) half 0 start ~370ns earlier
            m1 = tmp.tile([128, U], f16, tag="m1")
            m2 = tmp.tile([128, U], f16, tag="m2")
            c_new = st.tile([128, U], f16, tag="c")
            dve = []
            for half in (0, 1):
                lo, hi = 128 * half, 128 * (half + 1)
                dve.append(
                    nc.vector.tensor_mul(m1[:, lo:hi], sfo[:, U + lo : U + hi], c_prev[:, lo:hi])
                )
            for half in (0, 1):
                lo, hi = 128 * half, 128 * (half + 1)
                dve.append(nc.vector.tensor_mul(m2[:, lo:hi], igate[:, lo:hi], tg[:, lo:hi]))
                dve.append(nc.vector.tensor_add(c_new[:, lo:hi], m1[:, lo:hi], m2[:, lo:hi]))


            if last:
                tch = tmp.tile([128, U], f32, tag="tc32")
                nc.scalar.activation(tch[:], c_new[:], AF.Tanh)
                h_out = tmp.tile([128, U], f32, tag="hout")
                nc.vector.tensor_mul(h_out[:], sfo[:, 0:U], tch[:])
                nc.sync.dma_start(out[:], h_out[:])
            else:
                # tail split into u-halves so transpose/copy/h-matmul of half 0
                # start while half 1 is still in ACT/DVE
                hTn = [None, None]
                for half in (0, 1):
                    lo, hi = 128 * half, 128 * (half + 1)
                    tch = tmp.tile([128, 128], f16, tag=f"tc{half}")
                    nc.scalar.activation(tch[:], c_new[:, lo:hi], AF.Tanh)
                    hh = tmp.tile([128, 128], f16, tag=f"hh{half}")
                    hm = nc.vector.tensor_mul(hh[:], sfo[:, lo:hi], tch[:])
                    pp = psum_t.tile([128, 128], f16, tag="pt")
                    nc.tensor.transpose(pp[:], hh[:], ident[:])
                    ht_new = st.tile([128, B_L], f16, tag=f"h{half}")
                    nc.vector.tensor_copy(
                        ht_new[:].bitcast(mybir.dt.uint32),
                        pp[:].bitcast(mybir.dt.uint32),
                    )
                    hTn[half] = ht_new
                hT0, hT1 = hTn
            c_prev = c_new

    nc.compile()
    return nc


def get_program(has_bias: bool = False):
    if has_bias not in _cached:
        _cached[has_bias] = _build_program(has_bias)
    return _cached[has_bias]


def _reorder_cols(w):
    # [f | o | g] -> [f | g | o]
    return np.concatenate([w[:, 0:U], w[:, 2 * U : 3 * U], w[:, U : 2 * U]], axis=1)


def make_in_maps(inputs):
    x_dynamic = np.asarray(inputs["x_dynamic"], dtype=np.float32)
    x_static = np.asarray(inputs["x_static"], dtype=np.float32)
    w_ih = np.ascontiguousarray(
        _reorder_cols(np.asarray(inputs["weight_ih"], dtype=np.float32))
    )
    w_hh = np.ascontiguousarray(
        _reorder_cols(np.asarray(inputs["weight_hh"], dtype=np.float32))
    )
    w_sh = np.ascontiguousarray(np.asarray(inputs["weight_sh"], dtype=np.float32))
    bias = np.ascontiguousarray(
        _reorder_cols(np.asarray(inputs["bias"], dtype=np.float32).reshape(1, 3 * U))
    )
    bias_s = np.ascontiguousarray(
        np.asarray(inputs["bias_s"], dtype=np.float32).reshape(1, U)
    )
    in_maps = []
    for i in range(NCORES):
        sl = slice(i * B_L, (i + 1) * B_L)
        in_maps.append(
            {
                "x_dynamic": np.ascontiguousarray(
                    x_dynamic[sl].reshape(B_L, T * DYN)
                ),
                "x_static": np.ascontiguousarray(x_static[sl]),
                "weight_ih": w_ih,
                "weight_hh": w_hh,
                "weight_sh": w_sh,
                "bias": bias,
                "bias_s": bias_s,
            }
        )
    return in_maps


def kernel(**inputs) -> np.ndarray:
    from concourse.bass_utils import run_bass_kernel_spmd

    has_bias = bool(np.any(np.asarray(inputs["bias"])))
    nc = get_program(has_bias)
    in_maps = make_in_maps(inputs)
    res = run_bass_kernel_spmd(nc, in_maps, core_ids=list(range(NCORES)))
    return np.concatenate([r["out"] for r in res.results], axis=0).astype(np.float32)

